# revision 1
# baseline (speedup 1.0000x reference)
"""BoxFilter kernel for Trainium2 (8 NeuronCores) — bf16 I/O, v4.

out[b,0,i,j] = sum_c sum_{|di|<=15,|dj|<=15} x[b,c,i+di,j+dj] (edge-clamped),
matching the reference cumsum+shifted-diff formulation (separable box sums).

Sharding: data-parallel over (batch, H-half) -> 8 shards. Host converts to
bf16 and builds a channel-interleaved [1056, 3, 2048] slab per core (16 halo
rows each side), so each 128-row s-tile is ONE contiguous 1.5 MB DMA.

Per-core pipeline per 128-row output tile (engine roles from HW microbench;
HBM floor ~48us, DVE scan floor ~36us, PE warm ~230ns/matmul):
  - one [128, 3W] HWDGE load per s-tile, rings alternating; output stores on
    the SWDGE queue; all loads issued upfront, pool bufs throttle
  - channel sum: c0+c1 on DVE (in-place, 2x bf16) for edge tiles; folded
    into the PE band matmuls for middle tiles, so DVE (scan-bound ~36us) and
    PE (~41us) stay balanced under the DMA floor
  - vertical 31-tap box: accumulating bf16 matmuls per 512-col PSUM bank
  - ACT copies PSUM (f32) into the zero-padded xp tile (pads zeroed once)
  - horizontal 31-tap box: one DVE tensor_tensor_scan per tile
    (fp32 state, bf16 output -> no separate downcast)
  - PE pre-warmed with dummy matmuls during the DMA fill (HAM clock gate)
"""

import numpy as np
import ml_dtypes

BF = ml_dtypes.bfloat16

R = 15
TAP = 2 * R + 1          # 31
B, C, H, W = 4, 3, 2048, 2048
HALF = H // 2            # 1024 output rows per core
S_ROWS = HALF + 32       # 1056 input rows per core (16-row halo each side)
N_CORES = 8
PAD_L = TAP              # left zero pad for the scan (31)
XP_W = PAD_L + W + R     # 2094
SCAN_N = W + R           # 2063 scan steps; out col j = scan[j + R]
P = 128
N_OUT_TILES = HALF // P  # 8
TAIL_ROWS = S_ROWS - N_OUT_TILES * P  # 32 valid rows in the 9th s-tile
MM_N = 512               # one PSUM bank
FOLD = {3, 4, 5, 6, 7, 8}  # c1 via PE for these s-tiles (keeps DVE scans-only mid-stream)

_CACHE = {}


def _band_matrices():
    # out row i of a 128-row tile needs halo'd input rows r = i+1 .. i+31
    # (r indexed within the [s_lo; s_hi] 256-row window). 0/1 exact in bf16.
    k = np.arange(P)[:, None]
    i = np.arange(P)[None, :]
    band_a = ((k >= i + 1) & (k <= i + TAP)).astype(BF)          # rows in s_lo
    band_b = ((k + P >= i + 1) & (k + P <= i + TAP)).astype(BF)  # rows in s_hi
    return band_a, band_b


def _build_kernel(tc, nc, out, xs, band_a_d, band_b_d, mybir, bass):
    from contextlib import ExitStack

    f32 = mybir.dt.float32
    bf16 = mybir.dt.bfloat16
    add = mybir.AluOpType.add
    sub = mybir.AluOpType.subtract

    with ExitStack() as ctx:
        const_pool = ctx.enter_context(tc.tile_pool(name="const", bufs=1))
        xc_pool = ctx.enter_context(tc.tile_pool(name="xc", bufs=9))
        xp_pool = ctx.enter_context(tc.tile_pool(name="xp", bufs=1))
        box_pool = ctx.enter_context(tc.tile_pool(name="box", bufs=3))
        psum_pool = ctx.enter_context(
            tc.tile_pool(name="psum", bufs=7, space=bass.MemorySpace.PSUM)
        )
        warm_pool = ctx.enter_context(
            tc.tile_pool(name="warm", bufs=1, space=bass.MemorySpace.PSUM)
        )

        band_a = const_pool.tile([P, P], bf16)
        band_b = const_pool.tile([P, P], bf16)
        nc.sync.dma_start(band_a[:], band_a_d)
        nc.sync.dma_start(band_b[:], band_b_d)

        # keep the PE HAM clock gate warm while the first DMAs land
        wps = warm_pool.tile([P, MM_N], f32)
        for r in range(24):
            nc.tensor.matmul(wps[:, 0:P], band_a[:], band_a[:],
                             start=True, stop=True, skip_group_check=True)


        # persistent xp buffers: zero pads once, rotate manually
        N_XP = 4
        xps = [xp_pool.tile([P, XP_W], f32, name=f"xp{i}") for i in range(N_XP)]
        for x_ in xps:
            nc.gpsimd.memset(x_[:, 0:PAD_L], 0.0)
            nc.gpsimd.memset(x_[:, PAD_L + W : XP_W], 0.0)

        def load_s(u):
            """Issue the DMA loads for s-tile u; returns the xc tile."""
            rows = P if u < N_OUT_TILES else TAIL_ROWS
            xc = xc_pool.tile([P, 3 * W], bf16)
            if rows < P:
                # tail: rows >= TAIL_ROWS hit zero band weights but must be
                # finite (NaN * 0 = NaN on the PE)
                nc.gpsimd.memset(xc[:], 0.0)
            if u < 2:
                # pipeline fill: split the 1.5MB across three queues
                nc.sync.dma_start(xc[:rows, 0:W], xs[0, P * u : P * u + rows, :])
                nc.scalar.dma_start(
                    xc[:rows, W : 2 * W], xs[1, P * u : P * u + rows, :])
                nc.gpsimd.dma_start(
                    xc[:rows, 2 * W : 3 * W], xs[2, P * u : P * u + rows, :])
            else:
                # split every tile's 1.5MB across BOTH rings (column halves)
                # so tiles arrive in consumption order every ~3.5us
                W2 = W // 2
                xcv = xc[:rows].rearrange("p (c w) -> p c w", c=3)
                for h, eng in ((0, nc.sync), (1, nc.scalar)):
                    eng.dma_start(
                        xcv[:, :, h * W2 : (h + 1) * W2],
                        xs[:, P * u : P * u + rows, h * W2 : (h + 1) * W2]
                        .rearrange("c p w -> p c w"),
                    )
            return xc

        def finish_s(u, xc):
            """Channel-sum step for s-tile u -> list of matmul operands."""
            rows = P if u < N_OUT_TILES else TAIL_ROWS
            c0 = xc[:, 0:W]
            c1 = xc[:, W : 2 * W]
            c2 = xc[:, 2 * W : 3 * W]
            if u in FOLD and rows == P:
                return [c0, c1, c2]
            nc.vector.tensor_add(c0[:rows], c0[:rows], c1[:rows])
            return [c0, c2]

        s_tiles = {0: finish_s(0, load_s(0)), 1: finish_s(1, load_s(1)),
                   2: finish_s(2, load_s(2))}
        for t in range(N_OUT_TILES):
            if t + 3 <= N_OUT_TILES:
                s_tiles[t + 3] = finish_s(t + 3, load_s(t + 3))
            lo_ops, hi_ops = s_tiles.pop(t), s_tiles[t + 1]

            xp = xps[t % N_XP]

            # vertical box: accumulating matmuls per 512-col PSUM bank,
            # grouped by stationary weight (2 LDWEIGHTS per tile)
            psums = []
            for nb in range(W // MM_N):
                ps = psum_pool.tile([P, MM_N], f32)
                psums.append(ps)
            n_ops = len(lo_ops) + len(hi_ops)
            k = 0
            for band, ops in ((band_a, lo_ops), (band_b, hi_ops)):
                for op in ops:
                    k += 1
                    for nb in range(W // MM_N):
                        cs = slice(MM_N * nb, MM_N * (nb + 1))
                        nc.tensor.matmul(
                            psums[nb][:], band[:], op[:, cs],
                            start=(k == 1), stop=(k == n_ops),
                        )
            for nb in range(W // MM_N):
                nc.scalar.copy(
                    xp[:, PAD_L + MM_N * nb : PAD_L + MM_N * (nb + 1)],
                    psums[nb][:],
                )

            box = box_pool.tile([P, SCAN_N], bf16)
            if t == 0:
                # ramp tile: chained half-scans split at col 1023 so the
                # first half depends only on ACT banks 0-1
                H0 = 1023
                with tc.high_priority():
                    nc.vector.tensor_tensor_scan(
                        box[:, 0:H0],
                        xp[:, PAD_L : PAD_L + H0],
                        xp[:, 0:H0],
                        0.0,
                        add,
                        sub,
                    )
                nc.gpsimd.dma_start(
                    out[P * t : P * (t + 1), 0 : H0 - R], box[:, R:H0])
                with tc.high_priority():
                    nc.vector.tensor_tensor_scan(
                        box[:, H0:SCAN_N],
                        xp[:, PAD_L + H0 : PAD_L + SCAN_N],
                        xp[:, H0:SCAN_N],
                        box[:, H0 - 1 : H0],
                        add,
                        sub,
                    )
                nc.gpsimd.dma_start(
                    out[P * t : P * (t + 1), H0 - R : W],
                    box[:, H0 : R + W])
            elif t < N_OUT_TILES - 1:
                with tc.high_priority():
                    nc.vector.tensor_tensor_scan(
                        box[:],
                        xp[:, PAD_L : PAD_L + SCAN_N],
                        xp[:, 0:SCAN_N],
                        0.0,
                        add,
                        sub,
                    )
                nc.gpsimd.dma_start(
                    out[P * t : P * (t + 1), :], box[:, R : R + W])
            else:
                # last tile: chained half-scans so the first half-store
                # departs ~2us before the second half finishes
                HN = SCAN_N // 2 + 8
                with tc.high_priority():
                    nc.vector.tensor_tensor_scan(
                        box[:, 0:HN],
                        xp[:, PAD_L : PAD_L + HN],
                        xp[:, 0:HN],
                        0.0,
                        add,
                        sub,
                    )
                nc.gpsimd.dma_start(
                    out[P * t : P * (t + 1), 0 : HN - R], box[:, R:HN])
                with tc.high_priority():
                    nc.vector.tensor_tensor_scan(
                        box[:, HN:SCAN_N],
                        xp[:, PAD_L + HN : PAD_L + SCAN_N],
                        xp[:, HN:SCAN_N],
                        box[:, HN - 1 : HN],
                        add,
                        sub,
                    )
                nc.sync.dma_start(
                    out[P * t : P * (t + 1), HN - R : W],
                    box[:, HN : R + W])


def _get_nc():
    if "nc" in _CACHE:
        return _CACHE["nc"]
    import concourse.bass as bass
    import concourse.tile as tile
    from concourse import bacc, mybir

    nc = bacc.Bacc(
        "TRN2", target_bir_lowering=False, debug=False, num_devices=N_CORES
    )
    bf16 = mybir.dt.bfloat16
    xs = nc.dram_tensor("xs", [C, S_ROWS, W], bf16, kind="ExternalInput")
    ba = nc.dram_tensor("band_a", [P, P], bf16, kind="ExternalInput")
    bb = nc.dram_tensor("band_b", [P, P], bf16, kind="ExternalInput")
    out = nc.dram_tensor("out", [HALF, W], bf16, kind="ExternalOutput")

    with tile.TileContext(nc) as tc:
        _build_kernel(tc, nc, out.ap(), xs.ap(), ba.ap(), bb.ap(), mybir, bass)
    nc.compile()
    _CACHE["nc"] = nc
    return nc


def _in_maps(x):
    band_a, band_b = _band_matrices()
    xb = x.astype(BF)
    maps = []
    for k in range(N_CORES):
        b, half = divmod(k, 2)
        h0 = half * HALF
        lo = h0 - 16  # global row of xs row 0
        g0, g1 = max(lo, 0), min(h0 + HALF + 16, H)
        xs = np.zeros((C, S_ROWS, W), BF)
        xs[:, g0 - lo : g1 - lo, :] = xb[b, :, g0:g1, :]
        maps.append({"xs": xs, "band_a": band_a, "band_b": band_b})
    return maps


def _run(x, trace=False, tmpdir=None):
    from concourse.bass_utils import run_bass_kernel_spmd

    nc = _get_nc()
    res = run_bass_kernel_spmd(
        nc, _in_maps(x), list(range(N_CORES)), trace=trace, tmpdir=tmpdir
    )
    out = np.empty((B, 1, H, W), np.float32)
    for k in range(N_CORES):
        b, half = divmod(k, 2)
        out[b, 0, half * HALF : (half + 1) * HALF, :] = (
            res.results[k]["out"].astype(np.float32)
        )
    return out, res


def kernel(x: np.ndarray) -> np.ndarray:
    x = np.ascontiguousarray(x, dtype=np.float32)
    assert x.shape == (B, C, H, W)
    return _run(x)[0]

